# revision 8
# baseline (speedup 1.0000x reference)
"""KNN (B=4, N=M=8192, C=3, k=16) Bass kernel for 8 trn2 NeuronCores.

Sharding: core c handles batch b=c//2, query rows [ (c%2)*4096, +4096 ).
Each core computes, for its 4096 queries, psum[n, m] = -dist2[n, m] via a
K=30 bf16 TensorE matmul (1 cycle/row -- 4x faster than fp32):
    -d2 = sum_c 2*x1_c*x2_c  - |x1|^2 - |x2|^2
with each f32 factor split 3-way into bf16 parts (h+m+l) and the six
product terms hh+hm+mh+hl+mm+lh kept, giving ~2^-24 relative accuracy
(verified: max 7.6e-6 absolute error vs the f32 reference formula).

Top-16 per row with exact indices and no full-stream index rescan:
  stage 1: per 1024-block max8 (values) + max_index (local idx) -> 64
           candidates (value, global index) per row
  stage 2: pack candidates into sortable u32 keys
               key = (mono32(value) & ~0x1FFF) | (8191 - gidx)
           where mono32 is the order-preserving unsigned map of f32.
           max8/match_replace chains on the keys produce the top-24
           (value, index) pairs directly -- no max_index over the full
           stream, no gather.  A parallel max8 chain on the raw f32
           candidates gives the exact top-16 values; ScalarE takes
           sqrt(-v) for the output distances.
  host:    decodes indices from keys, flags rows where the 13-bit
           mantissa steal could have reordered near-ties (adjacent key
           value-parts equal), where a 1024-block overflowed 8 winners,
           or NaN/dup anomalies -- and recomputes those rows exactly
           (vectorized numpy, ~2% of rows).
"""

import numpy as np

import concourse.bass as bass  # noqa: F401  (engine classes register)
import concourse.bacc as bacc
from concourse import mybir, tile
from concourse.bass_utils import run_bass_kernel_spmd

B, N, M, C, K = 4, 8192, 8192, 3, 16
NCORES = 8
NLOC = B * N // NCORES      # 4096 query rows per core
P = 128                     # partition dim (queries per tile)
MB = 512                    # matmul moving-free chunk (one PSUM bank)
NMB = M // MB               # 16 chunks
SB = 1024                   # stage-1 max8 block size
NSB = M // SB               # 8 blocks
NEG_FILL = -3.0e38
KR = 30                     # 6 split-product terms x 5 features, bf16
IDXM = 0x1FFF               # 13-bit index field in keys
OC = 16 + 24 + 64           # pack: 16 f32 vals | 24 u32 keys | 64 f32 cand

_cached_nc = {}


def build(nt=NLOC // P):
    """Build + compile the SPMD program (nt row-tiles of 128 queries)."""
    if nt in _cached_nc:
        return _cached_nc[nt]
    f32 = mybir.dt.float32
    bf16 = mybir.dt.bfloat16
    u32 = mybir.dt.uint32
    i32 = mybir.dt.int32
    ALU = mybir.AluOpType
    nc = bacc.Bacc("TRN2", target_bir_lowering=False, debug=False,
                   num_devices=NCORES)
    # single packed input / output tensors: every extra PJRT operand costs
    # ~8 extra axon shard round-trips (~100 ms) per call, dwarfing exec time
    # bf16 payloads are packed two-per-f32-slot and bitcast device-side
    A1, R2, OFF = KR * NLOC // 2, KR * M // 2, P * 64
    flat_d = nc.dram_tensor("flat", [A1 + R2 + OFF], f32,
                            kind="ExternalInput")
    out_d = nc.dram_tensor("out", [nt, P, OC], u32, kind="ExternalOutput")
    a1_d = flat_d[0:A1].bitcast(bf16).rearrange("(a b) -> a b", b=NLOC)
    r2_d = flat_d[A1:A1 + R2].bitcast(bf16).rearrange("(a b) -> a b", b=M)
    off_d = flat_d[A1 + R2:A1 + R2 + OFF].rearrange("(a b) -> a b", b=64)

    with tile.TileContext(nc) as tc:
        with (
            tc.tile_pool(name="const", bufs=1) as constp,
            tc.tile_pool(name="psum", bufs=2, space="PSUM") as psump,
            tc.tile_pool(name="work", bufs=2) as workp,
            tc.tile_pool(name="outp", bufs=3) as outp,
        ):
            r2_sb = constp.tile([KR, M], bf16)
            nc.sync.dma_start(out=r2_sb[:], in_=r2_d)
            a1_sb = constp.tile([KR, NLOC], bf16)
            nc.sync.dma_start(out=a1_sb[:], in_=a1_d)
            # offp[c] = 8191 - (c//8)*1024  (per-candidate inverted block base)
            offp_sb = constp.tile([P, 64], u32)
            nc.sync.dma_start(out=offp_sb[:], in_=off_d.bitcast(u32))

            for t in range(nt):
                # ---- distances:  neg[p, m] = -dist2[p, m]  (fp32r matmul)
                neg = workp.tile([P, M], f32, tag="neg")
                for j0 in range(NMB // 4):
                    ps = psump.tile([P, 4 * MB], f32, tag="ps")
                    for j1 in range(4):
                        j = j0 * 4 + j1
                        nc.tensor.matmul(
                            ps[:, j1 * MB:(j1 + 1) * MB],
                            a1_sb[:, t * P:(t + 1) * P],
                            r2_sb[:, j * MB:(j + 1) * MB],
                            start=True, stop=True,
                        )
                    nc.scalar.copy(out=neg[:, j0 * 4 * MB:(j0 + 1) * 4 * MB],
                                   in_=ps[:])

                pack = outp.tile([P, OC], u32, tag="pack")
                cand = pack[:, 40:104].bitcast(f32)   # 64 candidate values
                lidx = workp.tile([P, 64], u32, tag="lidx")

                # ---- stage 1: per-block top-8 values + local indices ----
                for b in range(NSB):
                    nc.vector.max(cand[:, b * 8:(b + 1) * 8],
                                  neg[:, b * SB:(b + 1) * SB])
                for b in range(NSB):
                    nc.vector.max_index(lidx[:, b * 8:(b + 1) * 8],
                                        cand[:, b * 8:(b + 1) * 8],
                                        neg[:, b * SB:(b + 1) * SB])

                # ---- stage 2: pack (value, index) into sortable u32 keys --
                scr = workp.tile([P, 64], u32, tag="scr")
                key = workp.tile([P, 64], u32, tag="key")
                # gidx' = 8191 - (blockbase + lidx) = offp - lidx
                nc.vector.tensor_tensor(out=scr[:], in0=offp_sb[:],
                                        in1=lidx[:], op=ALU.subtract)
                # mono32(v) = bits ^ ((bits>>31) | 0x80000000)
                mono = workp.tile([P, 64], u32, tag="mono")
                nc.vector.tensor_scalar(out=mono[:].bitcast(i32),
                                        in0=cand[:].bitcast(i32),
                                        scalar1=31, scalar2=-2147483648,
                                        op0=ALU.arith_shift_right,
                                        op1=ALU.bitwise_or)
                nc.vector.tensor_tensor(out=mono[:], in0=cand[:].bitcast(u32),
                                        in1=mono[:], op=ALU.bitwise_xor)
                # key = (mono & ~0x1FFF) | gidx'
                nc.vector.tensor_scalar(out=key[:], in0=mono[:],
                                        scalar1=~IDXM & 0xFFFFFFFF,
                                        scalar2=None, op0=ALU.bitwise_and)
                nc.vector.tensor_tensor(out=key[:], in0=key[:], in1=scr[:],
                                        op=ALU.bitwise_or)

                # ---- key merge: top-24 (value,index) pairs, u32 order ----
                krep = workp.tile([P, 64], u32, tag="krep")
                nc.vector.max(pack[:, 16:24], key[:])
                nc.vector.match_replace(krep[:], pack[:, 16:24], key[:], 0.0)
                nc.vector.max(pack[:, 24:32], krep[:])
                nc.vector.match_replace(key[:], pack[:, 24:32], krep[:], 0.0)
                nc.vector.max(pack[:, 32:40], key[:])

                # ---- value merge: exact f32 top-16, then sqrt(-v) ----
                crep = workp.tile([P, 64], f32, tag="crep")
                v16 = workp.tile([P, K], f32, tag="v16")
                nc.vector.max(v16[:, 0:8], cand[:])
                nc.vector.match_replace(crep[:], v16[:, 0:8], cand[:],
                                        NEG_FILL)
                nc.vector.max(v16[:, 8:16], crep[:])
                nc.scalar.activation(
                    pack[:, 0:K].bitcast(f32), v16[:],
                    mybir.ActivationFunctionType.Sqrt,
                    scale=-1.0,
                )
                nc.sync.dma_start(out=out_d[t], in_=pack[:])

    nc.compile()
    _cached_nc[nt] = nc
    return nc


def _split3(x):
    import ml_dtypes
    bf = ml_dtypes.bfloat16
    h = x.astype(bf).astype(np.float32)
    m = (x - h).astype(bf).astype(np.float32)
    l = (x - h - m).astype(bf).astype(np.float32)
    return h, m, l


def make_in_maps(xyz1, xyz2):
    import ml_dtypes
    bf = ml_dtypes.bfloat16
    offp = (8191 - (np.arange(64, dtype=np.uint32) // 8) * 1024)
    offp = np.broadcast_to(offp, (P, 64)).copy().view(np.float32)
    in_maps = []
    for c in range(NCORES):
        b, h = c // 2, c % 2
        x1 = xyz1[b, h * NLOC:(h + 1) * NLOC]        # [NLOC, 3]
        x2 = xyz2[b]                                  # [M, 3]
        u = np.empty((5, NLOC), np.float32)          # [2*x1 | |x1|^2 | 1]
        u[0:3] = 2.0 * x1.T
        u[3] = (x1 * x1).sum(-1)
        u[4] = 1.0
        v = np.empty((5, M), np.float32)             # [x2 | -1 | -|x2|^2]
        v[0:3] = x2.T
        v[3] = -1.0
        v[4] = -(x2 * x2).sum(-1)
        uh, um, ul = _split3(u)
        vh, vm, vl = _split3(v)
        # term order: hh, hm, mh, hl, mm, lh
        a1t = np.concatenate([uh, uh, um, uh, um, ul]).astype(bf)  # [30, NLOC]
        r2 = np.concatenate([vh, vm, vh, vl, vm, vh]).astype(bf)   # [30, M]
        in_maps.append({
            "flat": np.concatenate([
                a1t.ravel().view(np.uint16).view(np.float32),
                r2.ravel().view(np.uint16).view(np.float32),
                offp.ravel(),
            ]).astype(np.float32),
        })
    return in_maps


def _fixup(vals, idx, suspect, xyz1, xyz2):
    """Host fallback: recompute suspect rows with the exact reference
    formula (stable top-k, NaN-first like lax.top_k)."""
    nrows = 0
    for b in range(vals.shape[0]):
        ns = np.flatnonzero(suspect[b])
        if ns.size == 0:
            continue
        nrows += ns.size
        x1 = xyz1[b, ns]                                     # [R, 3]
        x2 = xyz2[b]                                         # [M, 3]
        d2 = (-2.0 * (x1 @ x2.T) + (x1 * x1).sum(-1)[:, None]
              + (x2 * x2).sum(-1)[None, :]).astype(np.float32)
        dist = np.sqrt(d2)
        key = np.where(np.isnan(dist), np.float32(-np.inf), dist)
        part = np.argpartition(key, 3 * K // 2, axis=1)[:, :3 * K // 2]
        pv = np.take_along_axis(key, part, axis=1)
        order = np.lexsort((part, pv), axis=1)[:, :K]
        sel = np.take_along_axis(part, order, axis=1)
        vals[b, ns] = np.take_along_axis(dist, sel, axis=1)
        idx[b, ns] = sel.astype(np.int32)
    return nrows


def run(xyz1, xyz2, **spmd_kwargs):
    nc = build()
    in_maps = make_in_maps(xyz1, xyz2)
    res = run_bass_kernel_spmd(nc, in_maps, list(range(NCORES)), **spmd_kwargs)
    vals = np.empty((B, N, K), np.float32)
    idx = np.empty((B, N, K), np.int32)
    suspect = np.empty((B, N), bool)
    for c in range(NCORES):
        b, h = c // 2, c % 2
        sl = slice(h * NLOC, (h + 1) * NLOC)
        buf = res.results[c]["out"].reshape(NLOC, OC)
        vals[b, sl] = np.ascontiguousarray(buf[:, 0:16]).view(np.float32)
        keys = buf[:, 16:40]
        cand = np.ascontiguousarray(buf[:, 40:104]).view(np.float32)
        idx[b, sl] = (IDXM - (keys[:, 0:K] & IDXM)).astype(np.int32)
        # --- suspect-row detection (host side, vectorized) ---
        kp = keys[:, 0:17] & ~np.uint32(IDXM)
        s = (kp[:, 1:] == kp[:, :-1]).any(-1)          # 13-bit-steal near-tie
        csort = np.sort(cand, axis=1)[:, ::-1]
        s |= cand[:, 7::8].max(-1) >= csort[:, 15]     # 1024-block overflow
        vv = vals[b, sl]
        s |= np.isnan(vv).any(-1)                      # NaN ordering
        ii = idx[b, sl]
        sidx = np.sort(ii, axis=-1)
        s |= (sidx[:, 1:] == sidx[:, :-1]).any(-1)     # dup idx
        s |= (ii >= M).any(-1) | (ii < 0).any(-1)
        suspect[b, sl] = s
    nfix = _fixup(vals, idx, suspect, xyz1, xyz2)
    return (vals, idx), res, nfix


def kernel(xyz1, xyz2, k):
    xyz1 = np.asarray(xyz1, dtype=np.float32)
    xyz2 = np.asarray(xyz2, dtype=np.float32)
    assert int(k) == K, f"kernel hardcodes k={K}, got {k}"
    assert xyz1.shape == (B, N, C) and xyz2.shape == (B, M, C)
    (vals, idx), _, _ = run(xyz1, xyz2)
    return vals, idx


# revision 12
# speedup vs baseline: 1.0113x; 1.0113x over previous
"""KNN (B=4, N=M=8192, C=3, k=16) Bass kernel for 8 trn2 NeuronCores.

Sharding: core c handles batch b=c//2, query rows [ (c%2)*4096, +4096 ).
Each core computes, for its 4096 queries, psum[n, m] = -dist2[n, m] via a
K=30 bf16 TensorE matmul (1 cycle/row -- 4x faster than fp32):
    -d2 = sum_c 2*x1_c*x2_c  - |x1|^2 - |x2|^2
with each f32 factor split 3-way into bf16 parts (h+m+l) and the six
product terms hh+hm+mh+hl+mm+lh kept, giving ~2^-24 relative accuracy
(verified: max 7.6e-6 absolute error vs the f32 reference formula).

Top-16 per row with exact indices and no full-stream index rescan:
  stage 1: per 1024-block max8 (values) + max_index (local idx) -> 64
           candidates (value, global index) per row
  stage 2: embed each candidate's 13-bit global index in the low
           mantissa bits of its f32 value:
               key = bitcast_f32((bits(value) & ~0x1FFF) | gidx)
           f32 max8/match_replace chains (exact for f32; the u32 path
           rounds through f32 on HW!) then produce the top-24
           (value, index) pairs directly -- no max_index over the full
           stream, no gather.  Since -d2 values are negative, a larger
           embedded index sorts later, matching the reference's
           ascending-index tie order.  A parallel max8 chain on the raw
           f32 candidates gives the exact top-16 values; ScalarE takes
           sqrt(-v) for the output distances.
  host:    decodes indices from keys, flags rows where the 13-bit
           mantissa steal could have reordered near-ties (adjacent key
           value-parts equal), where a 1024-block overflowed 8 winners,
           or NaN/dup anomalies -- and recomputes those rows exactly
           (vectorized numpy, ~2% of rows).
"""

import numpy as np

import concourse.bass as bass  # noqa: F401  (engine classes register)
import concourse.bacc as bacc
from concourse import mybir, tile
from concourse.bass_utils import run_bass_kernel_spmd

B, N, M, C, K = 4, 8192, 8192, 3, 16
NCORES = 8
NLOC = B * N // NCORES      # 4096 query rows per core
P = 128                     # partition dim (queries per tile)
MB = 512                    # matmul moving-free chunk (one PSUM bank)
NMB = M // MB               # 16 chunks
SB = 1024                   # stage-1 max8 block size
NSB = M // SB               # 8 blocks
NEG_FILL = -3.0e38
KR = 30                     # 6 split-product terms x 5 features, bf16
IDXM = 0x1FFF               # 13-bit index field in keys
OC = 16 + 24 + 64           # pack: 16 f32 vals | 24 u32 keys | 64 f32 cand

_cached_nc = {}


def build(nt=NLOC // P):
    """Build + compile the SPMD program (nt row-tiles of 128 queries)."""
    if nt in _cached_nc:
        return _cached_nc[nt]
    f32 = mybir.dt.float32
    bf16 = mybir.dt.bfloat16
    u32 = mybir.dt.uint32
    i32 = mybir.dt.int32
    ALU = mybir.AluOpType
    nc = bacc.Bacc("TRN2", target_bir_lowering=False, debug=False,
                   num_devices=NCORES)
    # single packed input / output tensors: every extra PJRT operand costs
    # ~8 extra axon shard round-trips (~100 ms) per call, dwarfing exec time
    # bf16 payloads are packed two-per-f32-slot and bitcast device-side
    A1, R2, OFF = KR * NLOC // 2, KR * M // 2, P * 64
    flat_d = nc.dram_tensor("flat", [A1 + R2 + OFF], f32,
                            kind="ExternalInput")
    out_d = nc.dram_tensor("out", [nt, P, OC], u32, kind="ExternalOutput")
    a1_d = flat_d[0:A1].bitcast(bf16).rearrange("(a b) -> a b", b=NLOC)
    r2_d = flat_d[A1:A1 + R2].bitcast(bf16).rearrange("(a b) -> a b", b=M)
    off_d = flat_d[A1 + R2:A1 + R2 + OFF].rearrange("(a b) -> a b", b=64)

    with tile.TileContext(nc) as tc:
        with (
            tc.tile_pool(name="const", bufs=1) as constp,
            tc.tile_pool(name="psum", bufs=2, space="PSUM") as psump,
            tc.tile_pool(name="work", bufs=2) as workp,
            tc.tile_pool(name="outp", bufs=3) as outp,
        ):
            r2_sb = constp.tile([KR, M], bf16)
            nc.sync.dma_start(out=r2_sb[:], in_=r2_d)
            a1_sb = constp.tile([KR, NLOC], bf16)
            nc.sync.dma_start(out=a1_sb[:], in_=a1_d)
            # offp[c] = 8191 - (c//8)*1024  (per-candidate inverted block base)
            offp_sb = constp.tile([P, 64], u32)
            nc.sync.dma_start(out=offp_sb[:], in_=off_d.bitcast(u32))

            for t in range(nt):
                # ---- distances:  neg[p, m] = -dist2[p, m]  (fp32r matmul)
                neg = workp.tile([P, M], f32, tag="neg")
                for j0 in range(NMB // 4):
                    ps = psump.tile([P, 4 * MB], f32, tag="ps")
                    for j1 in range(4):
                        j = j0 * 4 + j1
                        nc.tensor.matmul(
                            ps[:, j1 * MB:(j1 + 1) * MB],
                            a1_sb[:, t * P:(t + 1) * P],
                            r2_sb[:, j * MB:(j + 1) * MB],
                            start=True, stop=True,
                        )
                    nc.scalar.copy(out=neg[:, j0 * 4 * MB:(j0 + 1) * 4 * MB],
                                   in_=ps[:])

                pack = outp.tile([P, OC], u32, tag="pack")
                cand = pack[:, 40:104].bitcast(f32)   # 64 candidate values
                lidx = workp.tile([P, 64], u32, tag="lidx")

                # ---- stage 1: per-block top-8 values + local indices ----
                for b in range(NSB):
                    nc.vector.max(cand[:, b * 8:(b + 1) * 8],
                                  neg[:, b * SB:(b + 1) * SB])
                for b in range(NSB):
                    nc.vector.max_index(lidx[:, b * 8:(b + 1) * 8],
                                        cand[:, b * 8:(b + 1) * 8],
                                        neg[:, b * SB:(b + 1) * SB])

                # ---- stage 2: embed gidx in low mantissa bits of value ----
                scr = workp.tile([P, 64], u32, tag="scr")
                key = workp.tile([P, 64], u32, tag="key")
                # gidx = blockbase + lidx
                nc.vector.tensor_tensor(out=scr[:], in0=offp_sb[:],
                                        in1=lidx[:], op=ALU.add)
                # key = (bits(value) & ~0x1FFF) | gidx
                nc.vector.tensor_scalar(out=key[:], in0=cand[:].bitcast(u32),
                                        scalar1=~IDXM & 0xFFFFFFFF,
                                        scalar2=None, op0=ALU.bitwise_and)
                nc.vector.tensor_tensor(out=key[:], in0=key[:], in1=scr[:],
                                        op=ALU.bitwise_or)

                # ---- key merge: top-24 (value,index) pairs, f32 order ----
                krep = workp.tile([P, 64], u32, tag="krep")
                kf = key[:].bitcast(f32)
                krf = krep[:].bitcast(f32)
                nc.vector.max(pack[:, 16:24].bitcast(f32), kf)
                nc.vector.match_replace(krf, pack[:, 16:24].bitcast(f32), kf,
                                        NEG_FILL)
                nc.vector.max(pack[:, 24:32].bitcast(f32), krf)
                nc.vector.match_replace(kf, pack[:, 24:32].bitcast(f32), krf,
                                        NEG_FILL)
                nc.vector.max(pack[:, 32:40].bitcast(f32), kf)

                # ---- value merge: exact f32 top-16, then sqrt(-v) ----
                crep = workp.tile([P, 64], f32, tag="crep")
                v16 = workp.tile([P, K], f32, tag="v16")
                nc.vector.max(v16[:, 0:8], cand[:])
                nc.vector.match_replace(crep[:], v16[:, 0:8], cand[:],
                                        NEG_FILL)
                nc.vector.max(v16[:, 8:16], crep[:])
                nc.scalar.activation(
                    pack[:, 0:K].bitcast(f32), v16[:],
                    mybir.ActivationFunctionType.Sqrt,
                    scale=-1.0,
                )
                nc.sync.dma_start(out=out_d[t], in_=pack[:])

    nc.compile()
    _cached_nc[nt] = nc
    return nc


def _split3(x):
    import ml_dtypes
    bf = ml_dtypes.bfloat16
    h = x.astype(bf).astype(np.float32)
    m = (x - h).astype(bf).astype(np.float32)
    l = (x - h - m).astype(bf).astype(np.float32)
    return h, m, l


def make_in_maps(xyz1, xyz2):
    import ml_dtypes
    bf = ml_dtypes.bfloat16
    offp = (np.arange(64, dtype=np.uint32) // 8) * 1024
    offp = np.broadcast_to(offp, (P, 64)).copy().view(np.float32)
    in_maps = []
    for c in range(NCORES):
        b, h = c // 2, c % 2
        x1 = xyz1[b, h * NLOC:(h + 1) * NLOC]        # [NLOC, 3]
        x2 = xyz2[b]                                  # [M, 3]
        u = np.empty((5, NLOC), np.float32)          # [2*x1 | |x1|^2 | 1]
        u[0:3] = 2.0 * x1.T
        u[3] = (x1 * x1).sum(-1)
        u[4] = 1.0
        v = np.empty((5, M), np.float32)             # [x2 | -1 | -|x2|^2]
        v[0:3] = x2.T
        v[3] = -1.0
        v[4] = -(x2 * x2).sum(-1)
        uh, um, ul = _split3(u)
        vh, vm, vl = _split3(v)
        # term order: hh, hm, mh, hl, mm, lh
        a1t = np.concatenate([uh, uh, um, uh, um, ul]).astype(bf)  # [30, NLOC]
        r2 = np.concatenate([vh, vm, vh, vl, vm, vh]).astype(bf)   # [30, M]
        in_maps.append({
            "flat": np.concatenate([
                a1t.ravel().view(np.uint16).view(np.float32),
                r2.ravel().view(np.uint16).view(np.float32),
                offp.ravel(),
            ]).astype(np.float32),
        })
    return in_maps


def _fixup(vals, idx, suspect, xyz1, xyz2):
    """Host fallback: recompute suspect rows with the exact reference
    formula (stable top-k, NaN-first like lax.top_k)."""
    nrows = 0
    for b in range(vals.shape[0]):
        ns = np.flatnonzero(suspect[b])
        if ns.size == 0:
            continue
        nrows += ns.size
        x1 = xyz1[b, ns]                                     # [R, 3]
        x2 = xyz2[b]                                         # [M, 3]
        d2 = (-2.0 * (x1 @ x2.T) + (x1 * x1).sum(-1)[:, None]
              + (x2 * x2).sum(-1)[None, :]).astype(np.float32)
        dist = np.sqrt(d2)
        key = np.where(np.isnan(dist), np.float32(-np.inf), dist)
        part = np.argpartition(key, 3 * K // 2, axis=1)[:, :3 * K // 2]
        pv = np.take_along_axis(key, part, axis=1)
        order = np.lexsort((part, pv), axis=1)[:, :K]
        sel = np.take_along_axis(part, order, axis=1)
        vals[b, ns] = np.take_along_axis(dist, sel, axis=1)
        idx[b, ns] = sel.astype(np.int32)
    return nrows


def run(xyz1, xyz2, **spmd_kwargs):
    nc = build()
    in_maps = make_in_maps(xyz1, xyz2)
    res = run_bass_kernel_spmd(nc, in_maps, list(range(NCORES)), **spmd_kwargs)
    vals = np.empty((B, N, K), np.float32)
    idx = np.empty((B, N, K), np.int32)
    suspect = np.empty((B, N), bool)
    for c in range(NCORES):
        b, h = c // 2, c % 2
        sl = slice(h * NLOC, (h + 1) * NLOC)
        buf = res.results[c]["out"].reshape(NLOC, OC)
        vals[b, sl] = np.ascontiguousarray(buf[:, 0:16]).view(np.float32)
        keys = buf[:, 16:40]
        cand = np.ascontiguousarray(buf[:, 40:104]).view(np.float32)
        idx[b, sl] = (keys[:, 0:K] & IDXM).astype(np.int32)
        # --- suspect-row detection (host side, vectorized) ---
        kp = keys[:, 0:17] & ~np.uint32(IDXM)
        s = (kp[:, 1:] == kp[:, :-1]).any(-1)          # 13-bit-steal near-tie
        csort = np.sort(cand, axis=1)[:, ::-1]
        s |= cand[:, 7::8].max(-1) >= csort[:, 15]     # 1024-block overflow
        vv = vals[b, sl]
        s |= np.isnan(vv).any(-1)                      # NaN ordering
        s |= (vv == 0.0).any(-1)                       # denormal-flush risk
        ii = idx[b, sl]
        sidx = np.sort(ii, axis=-1)
        s |= (sidx[:, 1:] == sidx[:, :-1]).any(-1)     # dup idx
        s |= (ii >= M).any(-1) | (ii < 0).any(-1)
        suspect[b, sl] = s
    nfix = _fixup(vals, idx, suspect, xyz1, xyz2)
    return (vals, idx), res, nfix


def kernel(xyz1, xyz2, k):
    xyz1 = np.asarray(xyz1, dtype=np.float32)
    xyz2 = np.asarray(xyz2, dtype=np.float32)
    assert int(k) == K, f"kernel hardcodes k={K}, got {k}"
    assert xyz1.shape == (B, N, C) and xyz2.shape == (B, M, C)
    (vals, idx), _, _ = run(xyz1, xyz2)
    return vals, idx


# revision 21
# speedup vs baseline: 1.0873x; 1.0752x over previous
"""KNN (B=4, N=M=8192, C=3, k=16) Bass kernel for 8 trn2 NeuronCores.

Sharding: core c handles batch b=c//2, query rows [ (c%2)*4096, +4096 ).
Each core computes, for its 4096 queries, psum[n, m] = -dist2[n, m] via a
K=30 bf16 TensorE matmul (1 cycle/row -- 4x faster than fp32):
    -d2 = sum_c 2*x1_c*x2_c  - |x1|^2 - |x2|^2
with each f32 factor split 3-way into bf16 parts (h+m+l) and the six
product terms hh+hm+mh+hl+mm+lh kept, giving ~2^-24 relative accuracy
(verified: max 7.6e-6 absolute error vs the f32 reference formula).

Top-16 per row with exact indices and no full-stream index rescan:
  stage 1: per 2048-block max8 (values) + max_index (local idx) -> 32
           candidates (value, global index) per row
  stage 2: (GpSimd) embed each candidate's 13-bit global index in the
           low mantissa bits of its f32 value:
               key = bitcast_f32((bits(value) & ~0x1FFF) | gidx)
           f32 max8/match_replace on the keys (exact for f32; the u32
           path rounds through f32 on HW!) produce the top-16
           (value, index) pairs directly -- no max_index over the full
           stream, no gather.  Since -d2 values are negative, a larger
           embedded index sorts later, matching the reference's
           ascending-index tie order.
  host:    receives [16 keys | 32 raw candidate f32] per row; derives
           exact sorted values (sqrt(-v)) and indices; flags rows where
           the 13-bit mantissa steal could have reordered near-ties
           (adjacent quantized values equal among top-17), where a
           2048-block overflowed 8 winners, or NaN/dup anomalies -- and
           recomputes those rows exactly (vectorized numpy, few %).
"""

import numpy as np

import concourse.bass as bass  # noqa: F401  (engine classes register)
import concourse.bacc as bacc
from concourse import mybir, tile
from concourse.bass_utils import run_bass_kernel_spmd

B, N, M, C, K = 4, 8192, 8192, 3, 16
NCORES = 8
NLOC = B * N // NCORES      # 4096 query rows per core
P = 128                     # partition dim (queries per tile)
MB = 512                    # matmul moving-free chunk (one PSUM bank)
NMB = M // MB               # 16 chunks
SB = 2048                   # stage-1 max8 block size
NSB = M // SB               # 4 blocks
NC_ = NSB * 8               # 32 candidates
NEG_FILL = -3.0e38
KR = 30                     # 6 split-product terms x 5 features, bf16
IDXM = 0x1FFF               # 13-bit index field in keys
OC = 16 + NC_               # pack: 16 u32 keys | 32 f32 cand

_cached_nc = {}


def build(nt=NLOC // P):
    """Build + compile the SPMD program (nt row-tiles of 128 queries)."""
    if nt in _cached_nc:
        return _cached_nc[nt]
    f32 = mybir.dt.float32
    bf16 = mybir.dt.bfloat16
    u32 = mybir.dt.uint32
    i32 = mybir.dt.int32
    ALU = mybir.AluOpType
    nc = bacc.Bacc("TRN2", target_bir_lowering=False, debug=False,
                   num_devices=NCORES)
    # single packed input / output tensors: every extra PJRT operand costs
    # ~8 extra axon shard round-trips (~100 ms) per call, dwarfing exec time
    # bf16 payloads are packed two-per-f32-slot and bitcast device-side
    A1, R2, OFF = KR * NLOC // 2, KR * M // 2, P * NC_
    flat_d = nc.dram_tensor("flat", [A1 + R2 + OFF], f32,
                            kind="ExternalInput")
    out_d = nc.dram_tensor("out", [nt, P, OC], u32, kind="ExternalOutput")
    a1_d = flat_d[0:A1].bitcast(bf16).rearrange("(a b) -> a b", b=NLOC)
    r2_d = flat_d[A1:A1 + R2].bitcast(bf16).rearrange("(a b) -> a b", b=M)
    off_d = flat_d[A1 + R2:A1 + R2 + OFF].rearrange("(a b) -> a b", b=NC_)

    with tile.TileContext(nc) as tc:
        with (
            tc.tile_pool(name="const", bufs=1) as constp,
            tc.tile_pool(name="psum", bufs=2, space="PSUM") as psump,
            tc.tile_pool(name="work", bufs=2) as workp,
            tc.tile_pool(name="outp", bufs=3) as outp,
        ):
            r2_sb = constp.tile([KR, M], bf16)
            nc.sync.dma_start(out=r2_sb[:], in_=r2_d)
            a1_sb = constp.tile([KR, NLOC], bf16)
            nc.sync.dma_start(out=a1_sb[:], in_=a1_d)
            # offp[c] = (c//8)*SB  (per-candidate block base)
            offp_sb = constp.tile([P, NC_], u32)
            nc.sync.dma_start(out=offp_sb[:], in_=off_d.bitcast(u32))

            for t in range(nt):
                # ---- distances:  neg[p, m] = -dist2[p, m]  (fp32r matmul)
                neg = workp.tile([P, M], f32, tag="neg")
                for j0 in range(NMB // 4):
                    ps = psump.tile([P, 4 * MB], f32, tag="ps")
                    for j1 in range(4):
                        j = j0 * 4 + j1
                        nc.tensor.matmul(
                            ps[:, j1 * MB:(j1 + 1) * MB],
                            a1_sb[:, t * P:(t + 1) * P],
                            r2_sb[:, j * MB:(j + 1) * MB],
                            start=True, stop=True,
                        )
                    nc.scalar.copy(out=neg[:, j0 * 4 * MB:(j0 + 1) * 4 * MB],
                                   in_=ps[:])

                pack = outp.tile([P, OC], u32, tag="pack")
                cand = pack[:, 16:16 + NC_].bitcast(f32)  # candidate values
                lidx = workp.tile([P, NC_], u32, tag="lidx")

                # ---- stage 1: per-block top-8 values + local indices ----
                for b in range(NSB):
                    nc.vector.max(cand[:, b * 8:(b + 1) * 8],
                                  neg[:, b * SB:(b + 1) * SB])
                for b in range(NSB):
                    nc.vector.max_index(lidx[:, b * 8:(b + 1) * 8],
                                        cand[:, b * 8:(b + 1) * 8],
                                        neg[:, b * SB:(b + 1) * SB])

                # ---- stage 2 (on GpSimd): embed gidx in low mantissa ----
                scr = workp.tile([P, NC_], u32, tag="scr")
                key = workp.tile([P, NC_], u32, tag="key")
                # gidx = blockbase + lidx  (Pool handles the integer add;
                # 32-bit bitwise ops are DVE-only per walrus)
                nc.gpsimd.tensor_tensor(out=scr[:], in0=offp_sb[:],
                                        in1=lidx[:], op=ALU.add)
                # key = (bits(value) & ~0x1FFF) | gidx
                nc.vector.tensor_scalar(out=key[:], in0=cand[:].bitcast(u32),
                                        scalar1=~IDXM & 0xFFFFFFFF,
                                        scalar2=None, op0=ALU.bitwise_and)
                nc.vector.tensor_tensor(out=key[:], in0=key[:], in1=scr[:],
                                        op=ALU.bitwise_or)

                # ---- key merge: top-16 (value,index) pairs, f32 order ----
                krep = workp.tile([P, NC_], u32, tag="krep")
                kf = key[:].bitcast(f32)
                krf = krep[:].bitcast(f32)
                nc.vector.max(pack[:, 0:8].bitcast(f32), kf)
                nc.vector.match_replace(krf, pack[:, 0:8].bitcast(f32), kf,
                                        NEG_FILL)
                nc.vector.max(pack[:, 8:16].bitcast(f32), krf)
                nc.sync.dma_start(out=out_d[t], in_=pack[:])

    nc.compile()
    _cached_nc[nt] = nc
    return nc


def _split3(x):
    import ml_dtypes
    bf = ml_dtypes.bfloat16
    h = x.astype(bf).astype(np.float32)
    m = (x - h).astype(bf).astype(np.float32)
    l = (x - h - m).astype(bf).astype(np.float32)
    return h, m, l


def make_in_maps(xyz1, xyz2):
    import ml_dtypes
    bf = ml_dtypes.bfloat16
    offp = (np.arange(NC_, dtype=np.uint32) // 8) * SB
    offp = np.broadcast_to(offp, (P, NC_)).copy().view(np.float32)
    in_maps = []
    for c in range(NCORES):
        b, h = c // 2, c % 2
        x1 = xyz1[b, h * NLOC:(h + 1) * NLOC]        # [NLOC, 3]
        x2 = xyz2[b]                                  # [M, 3]
        u = np.empty((5, NLOC), np.float32)          # [2*x1 | |x1|^2 | 1]
        u[0:3] = 2.0 * x1.T
        u[3] = (x1 * x1).sum(-1)
        u[4] = 1.0
        v = np.empty((5, M), np.float32)             # [x2 | -1 | -|x2|^2]
        v[0:3] = x2.T
        v[3] = -1.0
        v[4] = -(x2 * x2).sum(-1)
        uh, um, ul = _split3(u)
        vh, vm, vl = _split3(v)
        # term order: hh, hm, mh, hl, mm, lh
        a1t = np.concatenate([uh, uh, um, uh, um, ul]).astype(bf)  # [30, NLOC]
        r2 = np.concatenate([vh, vm, vh, vl, vm, vh]).astype(bf)   # [30, M]
        in_maps.append({
            "flat": np.concatenate([
                a1t.ravel().view(np.uint16).view(np.float32),
                r2.ravel().view(np.uint16).view(np.float32),
                offp.ravel(),
            ]).astype(np.float32),
        })
    return in_maps


def _fixup(vals, idx, suspect, xyz1, xyz2):
    """Host fallback: recompute suspect rows with the exact reference
    formula (stable top-k, NaN-first like lax.top_k)."""
    nrows = 0
    for b in range(vals.shape[0]):
        ns = np.flatnonzero(suspect[b])
        if ns.size == 0:
            continue
        nrows += ns.size
        x1 = xyz1[b, ns]                                     # [R, 3]
        x2 = xyz2[b]                                         # [M, 3]
        d2 = (-2.0 * (x1 @ x2.T) + (x1 * x1).sum(-1)[:, None]
              + (x2 * x2).sum(-1)[None, :]).astype(np.float32)
        dist = np.sqrt(d2)
        key = np.where(np.isnan(dist), np.float32(-np.inf), dist)
        part = np.argpartition(key, 3 * K // 2, axis=1)[:, :3 * K // 2]
        pv = np.take_along_axis(key, part, axis=1)
        order = np.lexsort((part, pv), axis=1)[:, :K]
        sel = np.take_along_axis(part, order, axis=1)
        vals[b, ns] = np.take_along_axis(dist, sel, axis=1)
        idx[b, ns] = sel.astype(np.int32)
    return nrows


def run(xyz1, xyz2, **spmd_kwargs):
    nc = build()
    in_maps = make_in_maps(xyz1, xyz2)
    res = run_bass_kernel_spmd(nc, in_maps, list(range(NCORES)), **spmd_kwargs)
    vals = np.empty((B, N, K), np.float32)
    idx = np.empty((B, N, K), np.int32)
    suspect = np.empty((B, N), bool)
    for c in range(NCORES):
        b, h = c // 2, c % 2
        sl = slice(h * NLOC, (h + 1) * NLOC)
        buf = res.results[c]["out"].reshape(NLOC, OC)
        keys = buf[:, 0:16]
        cand = np.ascontiguousarray(buf[:, 16:16 + NC_]).view(np.float32)
        idx[b, sl] = (keys & IDXM).astype(np.int32)
        # exact values: top-16 of cand (desc), dist = sqrt(-v)
        csort = np.sort(cand, axis=1)[:, ::-1]
        with np.errstate(invalid="ignore"):
            vals[b, sl] = np.sqrt(-csort[:, 0:K])
        # --- suspect-row detection (host side, vectorized) ---
        kp = csort[:, 0:K + 1].view(np.uint32) & ~np.uint32(IDXM)
        s = (kp[:, 1:] == kp[:, :-1]).any(-1)          # 13-bit-steal near-tie
        s |= cand[:, 7::8].max(-1) >= csort[:, 15]     # stage-1 block overflow
        vv = vals[b, sl]
        s |= np.isnan(vv).any(-1)                      # NaN ordering
        s |= (vv == 0.0).any(-1)                       # denormal-flush risk
        ii = idx[b, sl]
        sidx = np.sort(ii, axis=-1)
        s |= (sidx[:, 1:] == sidx[:, :-1]).any(-1)     # dup idx
        s |= (ii >= M).any(-1) | (ii < 0).any(-1)
        suspect[b, sl] = s
    nfix = _fixup(vals, idx, suspect, xyz1, xyz2)
    return (vals, idx), res, nfix


def kernel(xyz1, xyz2, k):
    xyz1 = np.asarray(xyz1, dtype=np.float32)
    xyz2 = np.asarray(xyz2, dtype=np.float32)
    assert int(k) == K, f"kernel hardcodes k={K}, got {k}"
    assert xyz1.shape == (B, N, C) and xyz2.shape == (B, M, C)
    (vals, idx), _, _ = run(xyz1, xyz2)
    return vals, idx
